# revision 1
# baseline (speedup 1.0000x reference)
"""ChannelMerger kernel for Trainium2, data-parallel over batch on 8 NeuronCores.

Reference computation (per batch b):
    pos       = layout + 0.2                              # [C, 2]
    loc[c,ij] = (2*pi/1.4) * (i * pos_x[c] + j * pos_y[c])   (i = ij>>5, j = ij&31)
    emb       = [cos(loc), sin(loc)]                      # [C, D=2048]
    scores    = emb @ heads.T                             # -> [O, C]
    weights   = softmax(scores, axis=C)
    out[b]    = weights @ x[b]                            # [O, T]

Device program (identical on all 8 cores, each owns 8 batches):
  phase 1 (replicated): embT [d, c] built directly in transposed layout via
    fractional-turn range reduction + ACT Sin; heads transposed on the PE;
    scoresT = embT.T @ headsT -> [c, o]; expT = exp(scoresT) (unnormalized
    softmax, f32r); per-o sums via ones-matmul; recip[o] = 1/sum.
  phase 2: out[b] = (expT.T @ x[b]) * recip[o]  -- fp32r matmuls, N-chunks of
    512 so each matmul stays inside one PSUM bank.
"""

import sys

for _p in ("/opt/trn_rl_repo", "/root/.axon_site/_ro/trn_rl_repo"):
    if _p not in sys.path:
        sys.path.append(_p)

import numpy as np

B, C, T = 64, 270, 2000
O, D = 270, 2048
N_CORES = 8
B_LOC = B // N_CORES          # 8 batches per core
NF = 32                       # fourier freqs per axis; NF*NF = 1024 = D//2
MARGIN = 0.2
WIDTH = 1.0 + 2.0 * MARGIN    # 1.4

# chunkings
C_CHUNKS = [(0, 128), (128, 128), (256, 14)]    # c (contraction) and o (output rows)
D_CHUNKS = 16                                   # 2048 / 128
IJ_CHUNKS = 8                                   # 1024 / 128
N_TILE = 512                                    # psum-bank-aligned t chunks
N_CHUNKS = [(0, 512), (512, 512), (1024, 512), (1536, 464)]

_cache = {}


def _build(repeat=1):
    import concourse.tile as tile
    from concourse import bacc, mybir
    from concourse.masks import make_identity

    F32 = mybir.dt.float32
    F32R = mybir.dt.float32r
    I32 = mybir.dt.int32
    ACT = mybir.ActivationFunctionType
    ALU = mybir.AluOpType
    TWO_PI = float(2.0 * np.pi)

    nc = bacc.Bacc("TRN2", target_bir_lowering=False, debug=False,
                   num_devices=N_CORES)

    x_ap = nc.dram_tensor("x", [B_LOC, C, T], F32, kind="ExternalInput").ap()
    lay_ap = nc.dram_tensor("layout", [C, 2], F32, kind="ExternalInput").ap()
    heads_ap = nc.dram_tensor("heads", [O, D], F32, kind="ExternalInput").ap()
    # ijc[:, k] = float((k*128 + p) >> 5) for k < 8; ijc[:, 8] = float(p & 31);
    # ijc[:, 9] = 1.0
    ijc_ap = nc.dram_tensor("ijc", [128, 10], F32, kind="ExternalInput").ap()
    out_ap = nc.dram_tensor("out", [B_LOC, O, T], F32, kind="ExternalOutput").ap()

    with tile.TileContext(nc) as tc:
      for _rep in range(repeat):
        with tc.tile_pool(name="const", bufs=1) as cpool, \
             tc.tile_pool(name="expT", bufs=1) as epool:

            ident = cpool.tile([128, 128], F32)
            make_identity(nc, ident[:])
            ijc = cpool.tile([128, 10], F32)
            nc.sync.dma_start(ijc[:], ijc_ap[:])

            # pos rows -> scaled turn coefficients u = (pos_x+0.2)/1.4, v likewise
            posx = cpool.tile([1, C], F32)
            posy = cpool.tile([1, C], F32)
            nc.sync.dma_start(posx[:], lay_ap[:, 0])
            nc.sync.dma_start(posy[:], lay_ap[:, 1])
            u_row = cpool.tile([1, C], F32)
            nc.vector.tensor_scalar(u_row[:], posx[:], MARGIN, 1.0 / WIDTH,
                                    ALU.add, ALU.mult)
            v_row = cpool.tile([1, C], F32)
            nc.vector.tensor_scalar(v_row[:], posy[:], MARGIN, 1.0 / WIDTH,
                                    ALU.add, ALU.mult)
            u_bc = cpool.tile([128, C], F32)
            nc.gpsimd.partition_broadcast(u_bc[:], u_row[:])
            v_bc = cpool.tile([128, C], F32)
            nc.gpsimd.partition_broadcast(v_bc[:], v_row[:])

            # long-lived phase-1 outputs
            expT = [epool.tile([128, C], F32R, tag=f"expT{i}", name=f"expT{i}") for i in range(3)]
            recip = epool.tile([128, 4], F32)

            # phase-2 pools allocated BEFORE the phase-1 pools so their SBUF
            # ranges never overlap: x loads then carry no anti-dependency on
            # phase-1 tiles and can stream from t=0.
            with tc.tile_pool(name="xin", bufs=5) as xpool, \
                 tc.tile_pool(name="oout", bufs=3) as opool:
              with tc.tile_pool(name="embT", bufs=1) as embpool, \
                   tc.tile_pool(name="headsT", bufs=1) as htpool:

                embT = [embpool.tile([128, C], F32R, tag=f"embT{i}", name=f"embT{i}")
                        for i in range(D_CHUNKS)]
                headsT = [htpool.tile([128, O], F32R, tag=f"headsT{i}", name=f"headsT{i}")
                          for i in range(D_CHUNKS)]

                # ---- transpose heads on the PE: headsT[dc][d, o] ----
                with tc.tile_pool(name="heads_in", bufs=2) as hpool, \
                     tc.tile_pool(name="tp_psum", bufs=6, space="PSUM") as tpp:
                    for oc, (o0, osz) in enumerate(C_CHUNKS):
                        hsb = hpool.tile([128, D], F32, tag="heads", name="heads")
                        nc.sync.dma_start(hsb[:osz, :],
                                          heads_ap[o0:o0 + osz, :])
                        for dc in range(D_CHUNKS):
                            pt = tpp.tile([128, 128], F32, tag="tp")
                            nc.tensor.transpose(
                                pt[:, :osz],
                                hsb[:osz, dc * 128:(dc + 1) * 128],
                                ident[:osz, :osz])
                            if dc % 3 == 2:
                                # ACT idles until the Sin chain produces; a
                                # third of the psum->sbuf copies go there to
                                # relieve DVE (the phase-1 throughput limit)
                                nc.scalar.activation(
                                    headsT[dc][:, o0:o0 + osz], pt[:, :osz],
                                    ACT.Copy)
                            else:
                                nc.vector.tensor_copy(
                                    headsT[dc][:, o0:o0 + osz], pt[:, :osz])

                # ---- embedding, transposed: embT[k][p, c] ----
                with tc.tile_pool(name="emb_work", bufs=3) as wpool:
                    # t2 = j*v is the same for every ij chunk (j = p & 31)
                    t2 = wpool.tile([128, C], F32, tag="t2", bufs=1)
                    nc.gpsimd.tensor_scalar(t2[:], v_bc[:], ijc[:, 8:9], None,
                                            ALU.mult)
                    for k in range(IJ_CHUNKS):
                        i_col = ijc[:, k:k + 1]
                        f = wpool.tile([128, C], F32, tag="f")
                        nc.vector.scalar_tensor_tensor(
                            f[:], u_bc[:], i_col, t2[:], ALU.mult, ALU.add)
                        # sin chunk: emb[:, 1024 + k*128 : ...] = sin(2*pi*f)
                        ki = wpool.tile([128, C], I32, tag="ki", bufs=2)
                        nc.vector.tensor_copy(ki[:], f[:])
                        kf = wpool.tile([128, C], F32, tag="kf", bufs=2)
                        nc.gpsimd.tensor_copy(kf[:], ki[:])
                        fs = wpool.tile([128, C], F32, tag="fs")
                        nc.vector.tensor_tensor(fs[:], f[:], kf[:], ALU.subtract)
                        nc.scalar.activation(embT[8 + k][:], fs[:], ACT.Sin,
                                             scale=TWO_PI)
                        # cos chunk: cos(2*pi*f) = sin(2*pi*(f+0.25))
                        g = wpool.tile([128, C], F32, tag="g")
                        nc.gpsimd.tensor_scalar(g[:], f[:], 0.25, None, ALU.add)
                        gi = wpool.tile([128, C], I32, tag="gi", bufs=2)
                        nc.vector.tensor_copy(gi[:], g[:])
                        gf = wpool.tile([128, C], F32, tag="gf", bufs=2)
                        nc.gpsimd.tensor_copy(gf[:], gi[:])
                        gs = wpool.tile([128, C], F32, tag="gs")
                        nc.vector.tensor_tensor(gs[:], g[:], gf[:], ALU.subtract)
                        nc.scalar.activation(embT[k][:], gs[:], ACT.Sin,
                                             scale=TWO_PI)

                # ---- scoresT = embT.T @ headsT ; expT = exp(scoresT) ----
                with tc.tile_pool(name="sc_psum", bufs=3, space="PSUM") as scp, \
                     tc.tile_pool(name="sum_psum", bufs=1, space="PSUM") as sup, \
                     tc.tile_pool(name="sum_work", bufs=1) as swp:
                    expF = [swp.tile([128, C], F32, tag=f"expF{i}",
                                     name=f"expF{i}") for i in range(3)]
                    for cc, (c0, csz) in enumerate(C_CHUNKS):
                        ps = scp.tile([128, O], F32, tag="sc")
                        for dc in range(D_CHUNKS):
                            nc.tensor.matmul(ps[:csz, :],
                                             embT[dc][:, c0:c0 + csz],
                                             headsT[dc][:],
                                             start=(dc == 0),
                                             stop=(dc == D_CHUNKS - 1))
                        nc.scalar.activation(expF[cc][:csz, :], ps[:csz, :],
                                             ACT.Exp)
                        nc.vector.tensor_copy(expT[cc][:csz, :],
                                              expF[cc][:csz, :])
                    # sums over c for each o-chunk (plain fp32), then recip
                    for oc, (o0, osz) in enumerate(C_CHUNKS):
                        ps = sup.tile([128, 1], F32, tag="sum")
                        for cc, (c0, csz) in enumerate(C_CHUNKS):
                            nc.tensor.matmul(ps[:osz, :],
                                             expF[cc][:csz, o0:o0 + osz],
                                             ijc[:csz, 9:10],
                                             start=(cc == 0), stop=(cc == 2))
                        nc.vector.reciprocal(recip[:osz, oc:oc + 1],
                                             ps[:osz, :])

              # ---- phase 2: out[b] = (expT.T @ x[b]) * recip ----
              # xin2 lives in the SBUF freed by the phase-1 pools; its loads
              # carry anti-deps on phase-1 tiles, which is fine because the
              # late batches are consumed late anyway. Early batches stream
              # from t=0 via the preallocated xin pool.
              with tc.tile_pool(name="mm_psum", bufs=4, space="PSUM") as mmp:
                  for b in range(B_LOC):
                      pool_b = xpool
                      xb = []
                      for cc, (c0, csz) in enumerate(C_CHUNKS):
                          xt = pool_b.tile([128, T], F32R, tag=f"x{cc}", name=f"x{cc}")
                          nc.sync.dma_start(
                              xt[:csz, :],
                              x_ap[b, c0:c0 + csz, :].bitcast(F32R))
                          xb.append(xt)
                      for oc, (o0, osz) in enumerate(C_CHUNKS):
                          # two half-width psum tiles per m-block: the scale
                          # copy of the first half overlaps the matmuls of
                          # the second, shortening the PE->DVE->DMA chain
                          ot = opool.tile([128, T], F32, tag="o")
                          for h, half in enumerate([N_CHUNKS[:2], N_CHUNKS[2:]]):
                              ph = mmp.tile([128, 1024], F32, tag="mm")
                              base = half[0][0]
                              for cc, (c0, csz) in enumerate(C_CHUNKS):
                                  for (n0, nsz) in half:
                                      nc.tensor.matmul(
                                          ph[:osz, n0 - base:n0 - base + nsz],
                                          expT[cc][:csz, o0:o0 + osz],
                                          xb[cc][:csz, n0:n0 + nsz],
                                          start=(cc == 0), stop=(cc == 2))
                              hw = min(1024, T - base)
                              nc.vector.tensor_scalar(
                                  ot[:osz, base:base + hw], ph[:osz, :hw],
                                  recip[:osz, oc:oc + 1], None, ALU.mult)
                          # SWDGE (gpsimd) queue: keeps result stores off the
                          # sync queue so they can't head-of-line-block x loads
                          nc.gpsimd.dma_start(out_ap[b, o0:o0 + osz, :],
                                              ot[:osz, :])

    nc.compile()
    return nc


def _ijc_const():
    p = np.arange(128)
    cols = [((k * 128 + p) >> 5).astype(np.float32) for k in range(IJ_CHUNKS)]
    cols.append((p & 31).astype(np.float32))
    cols.append(np.ones(128, np.float32))
    return np.stack(cols, axis=1)


def get_nc(repeat=1):
    key = f"nc{repeat}"
    if key not in _cache:
        _cache[key] = _build(repeat)
    return _cache[key]


def kernel(x, layout, heads):
    from concourse.bass_utils import run_bass_kernel_spmd

    assert x.shape == (B, C, T) and layout.shape == (C, 2)
    assert heads.shape == (O, D)
    nc = get_nc()
    ijc = _ijc_const()
    in_maps = [
        {
            "x": np.ascontiguousarray(x[m * B_LOC:(m + 1) * B_LOC]),
            "layout": np.ascontiguousarray(layout.astype(np.float32)),
            "heads": np.ascontiguousarray(heads.astype(np.float32)),
            "ijc": ijc,
        }
        for m in range(N_CORES)
    ]
    res = run_bass_kernel_spmd(nc, in_maps, list(range(N_CORES)))
    out = np.concatenate([res.results[m]["out"] for m in range(N_CORES)], axis=0)
    return out.astype(np.float32)



# revision 2
# speedup vs baseline: 19.0437x; 19.0437x over previous
"""ChannelMerger v2: batched-op phase 1 + bf16 phase 2.

Host staging (untimed): x -> bf16 channel-major [C, B_LOC*T] per core;
heads -> bf16 headsT packed [128, 16, O]; out bf16 [O, B_LOC*T] -> host
transpose/cast back to [B, O, T] f32.

Device program per core (replicated, B_LOC=8 batches):
  phase 1: u,v from layout; f_all[p, k, c] = i(k,p)*u[c] + j(p)*v[c] via two
    big DVE ops; frac via mod; two big ACT Sin calls -> cos_all/sin_all bf16;
    scores psum[c,o] = sum_k embT_k.T @ headsT_k (bf16); expT = exp (bf16);
    sums via ones-matmul; recip = 1/sum.
  phase 2: per batch (2000-col block): 3 bf16 x loads; 18 bf16 matmuls into
    [osz, 1024/976] psum; scale-by-recip copies (DVE/gpsimd alternating) to
    bf16 out tiles; stores on scalar-engine HWDGE queue.
"""

import sys

for _p in ("/opt/trn_rl_repo", "/root/.axon_site/_ro/trn_rl_repo"):
    if _p not in sys.path:
        sys.path.append(_p)

import numpy as np
import ml_dtypes

BF16 = ml_dtypes.bfloat16

B, C, T = 64, 270, 2000
O, D = 270, 2048
N_CORES = 8
B_LOC = B // N_CORES
NF = 32
MARGIN = 0.2
WIDTH = 1.4
KC = 8                           # ij chunks per half (8 * 128 = 1024 = D/2)
C_CHUNKS = [(0, 128), (128, 128), (256, 14)]
HALF = 8000                      # x/out processed in two 8000-column halves
PBLK = 2048                      # psum block width (4 banks); matmuls 512-wide
BLT = B_LOC * T

_cache = {}


def _build(repeat=1):
    import concourse.tile as tile
    from concourse import bacc, mybir

    F32 = mybir.dt.float32
    BF = mybir.dt.bfloat16
    I32 = mybir.dt.int32
    ACT = mybir.ActivationFunctionType
    ALU = mybir.AluOpType
    TWO_PI = float(2.0 * np.pi)

    nc = bacc.Bacc("TRN2", target_bir_lowering=False, debug=False,
                   num_devices=N_CORES)

    xc_ap = nc.dram_tensor("xc", [C, BLT], BF, kind="ExternalInput").ap()
    lay_ap = nc.dram_tensor("layout", [C, 2], F32, kind="ExternalInput").ap()
    # headsT packed: hT[p, k, o] = heads[o, k*128 + p]  (k = 0..15)
    ht_ap = nc.dram_tensor("headsT", [128, 2 * KC, O], BF,
                           kind="ExternalInput").ap()
    # tab[:, 0] = p & 31; tab[:, 1] = 1.0; tab[:, 2+k] = (k*128 + p) >> 5
    tab_ap = nc.dram_tensor("tab", [128, 2 + KC], F32, kind="ExternalInput").ap()
    out_ap = nc.dram_tensor("out", [O, BLT], BF, kind="ExternalOutput").ap()

    with tile.TileContext(nc) as tc:
      for _rep in range(repeat):
        with tc.tile_pool(name="const", bufs=1) as cpool, \
             tc.tile_pool(name="expTp", bufs=1) as epool, \
             tc.tile_pool(name="xin", bufs=2) as xpool, \
             tc.tile_pool(name="oout", bufs=1) as opool:

            tab = cpool.tile([128, 2 + KC], F32)
            nc.sync.dma_start(tab[:], tab_ap[:])
            headsT = cpool.tile([128, 2 * KC, O], BF)
            nc.sync.dma_start(headsT[:], ht_ap[:])

            posx = cpool.tile([1, C], F32)
            posy = cpool.tile([1, C], F32)
            nc.sync.dma_start(posx[:], lay_ap[:, 0])
            nc.sync.dma_start(posy[:], lay_ap[:, 1])
            u_row = cpool.tile([1, C], F32)
            nc.vector.tensor_scalar(u_row[:], posx[:], MARGIN, 1.0 / WIDTH,
                                    ALU.add, ALU.mult)
            v_row = cpool.tile([1, C], F32)
            nc.vector.tensor_scalar(v_row[:], posy[:], MARGIN, 1.0 / WIDTH,
                                    ALU.add, ALU.mult)
            u_bc = cpool.tile([128, C], F32)
            nc.gpsimd.partition_broadcast(u_bc[:], u_row[:])
            v_bc = cpool.tile([128, C], F32)
            nc.gpsimd.partition_broadcast(v_bc[:], v_row[:])

            expT = [epool.tile([128, O], BF, tag=f"expT{i}", name=f"expT{i}")
                    for i in range(3)]

            with tc.tile_pool(name="ph1", bufs=1) as wpool:
                # t2[p, c] = j(p) * v[c]
                t2 = wpool.tile([128, C], F32)
                nc.gpsimd.tensor_scalar(t2[:], v_bc[:], tab[:, 0:1], None,
                                        ALU.mult)
                # f_all[p, k, c] = i(k, p) * u[c]  (one big DVE op)
                f_all = wpool.tile([128, KC, C], F32)
                nc.vector.tensor_tensor(
                    f_all[:],
                    tab[:, 2:2 + KC].unsqueeze(2).broadcast_to([128, KC, C]),
                    u_bc[:].unsqueeze(1).broadcast_to([128, KC, C]),
                    ALU.mult)
                # f_all += t2
                nc.vector.tensor_tensor(
                    f_all[:], f_all[:],
                    t2[:].unsqueeze(1).broadcast_to([128, KC, C]),
                    ALU.add)
                # sin half: f - round(f) in [-.5,.5] on hw, sin(2*pi*frac)
                fi = wpool.tile([128, KC, C], I32, tag="fi")
                nc.vector.tensor_copy(fi[:], f_all[:])
                ff = wpool.tile([128, KC, C], F32, tag="ff")
                nc.gpsimd.tensor_copy(ff[:], fi[:])
                fs = wpool.tile([128, KC, C], F32, tag="fs")
                nc.vector.tensor_tensor(fs[:], f_all[:], ff[:], ALU.subtract)
                sin_all = wpool.tile([128, KC, C], BF)
                nc.scalar.activation(sin_all[:], fs[:], ACT.Sin, scale=TWO_PI)
                # cos half: frac(f + 0.25); reuse buffers of f_all/fi/ff/fs
                g = wpool.tile([128, KC, C], F32, tag="f_all")
                nc.vector.tensor_scalar(g[:], fs[:], 0.25, None, ALU.add)
                gi = wpool.tile([128, KC, C], I32, tag="fi")
                nc.vector.tensor_copy(gi[:], g[:])
                gf = wpool.tile([128, KC, C], F32, tag="ff")
                nc.gpsimd.tensor_copy(gf[:], gi[:])
                gs = wpool.tile([128, KC, C], F32, tag="fs")
                nc.vector.tensor_tensor(gs[:], g[:], gf[:], ALU.subtract)
                cos_all = wpool.tile([128, KC, C], BF)
                nc.scalar.activation(cos_all[:], gs[:], ACT.Sin, scale=TWO_PI)

                ones = wpool.tile([128, 1], BF)
                nc.vector.memset(ones[:], 1.0)

                with tc.tile_pool(name="sc_psum", bufs=2, space="PSUM") as scp, \
                     tc.tile_pool(name="sum_psum", bufs=1, space="PSUM") as sup:
                    for cc, (c0, csz) in enumerate(C_CHUNKS):
                        ps = scp.tile([128, O], F32, tag="sc")
                        for k in range(KC):
                            nc.tensor.matmul(ps[:csz, :],
                                             cos_all[:, k, c0:c0 + csz],
                                             headsT[:, k, :],
                                             start=(k == 0), stop=False)
                        for k in range(KC):
                            nc.tensor.matmul(ps[:csz, :],
                                             sin_all[:, k, c0:c0 + csz],
                                             headsT[:, KC + k, :],
                                             start=False, stop=(k == KC - 1))
                        nc.scalar.activation(expT[cc][:csz, :], ps[:csz, :],
                                             ACT.Exp)
                    # denominators as a row: sums[1, o] = sum_c exp[c, o]
                    ps = sup.tile([1, O], F32, tag="sum")
                    for cc, (c0, csz) in enumerate(C_CHUNKS):
                        nc.tensor.matmul(ps[:, :], ones[:csz, :],
                                         expT[cc][:csz, :],
                                         start=(cc == 0), stop=(cc == 2))
                    rrow = wpool.tile([1, O], F32)
                    nc.vector.reciprocal(rrow[:], ps[:, :])
                    rbc = wpool.tile([128, O], F32)
                    nc.gpsimd.partition_broadcast(rbc[:], rrow[:])
                    # fold softmax normalization into the weights
                    for cc, (c0, csz) in enumerate(C_CHUNKS):
                        nc.vector.tensor_tensor(expT[cc][:csz, :],
                                                expT[cc][:csz, :],
                                                rbc[:csz, :], ALU.mult)

            # ---- phase 2: two 8000-column halves, big DMAs ----
            with tc.tile_pool(name="mm_psum", bufs=2, space="PSUM") as mmp:
                for h in range(2):
                    base = h * HALF
                    xb = []
                    for cc, (c0, csz) in enumerate(C_CHUNKS):
                        xt = xpool.tile([128, HALF], BF, tag=f"x{cc}",
                                        name=f"x{cc}")
                        nc.sync.dma_start(xt[:csz, :],
                                          xc_ap[c0:c0 + csz, base:base + HALF])
                        xb.append(xt)
                    for oc, (o0, osz) in enumerate(C_CHUNKS):
                        ot = opool.tile([128, HALF], BF, tag=f"o{oc}",
                                        name=f"o{oc}")
                        for p0 in range(0, HALF, PBLK):
                            psz = min(PBLK, HALF - p0)
                            ph = mmp.tile([128, PBLK], F32, tag="mm")
                            for cc, (c0, csz) in enumerate(C_CHUNKS):
                                for s0 in range(0, psz, 512):
                                    ssz = min(512, psz - s0)
                                    nc.tensor.matmul(
                                        ph[:osz, s0:s0 + ssz],
                                        expT[cc][:csz, o0:o0 + osz],
                                        xb[cc][:csz, p0 + s0:p0 + s0 + ssz],
                                        start=(cc == 0), stop=(cc == 2))
                            if (oc + p0 // PBLK) % 2 == 0:
                                nc.vector.tensor_copy(ot[:osz, p0:p0 + psz],
                                                      ph[:osz, :psz])
                            else:
                                nc.scalar.activation(ot[:osz, p0:p0 + psz],
                                                     ph[:osz, :psz], ACT.Copy)
                        nc.scalar.dma_start(
                            out_ap[o0:o0 + osz, base:base + HALF], ot[:osz, :])

    nc.compile()
    return nc


def _tab_const():
    p = np.arange(128)
    cols = [(p & 31).astype(np.float32), np.ones(128, np.float32)]
    cols += [((k * 128 + p) >> 5).astype(np.float32) for k in range(KC)]
    return np.stack(cols, axis=1)


def _stage_heads(heads):
    # heads [O, D] f32 -> hT[p, k, o] = heads[o, k*128+p], bf16
    hT = heads.T.astype(BF16)                     # [D, O]
    return np.ascontiguousarray(
        hT.reshape(2 * KC, 128, O).transpose(1, 0, 2))


def _stage_x(x_core):
    # x_core [B_LOC, C, T] f32 -> [C, B_LOC*T] bf16
    return np.ascontiguousarray(
        x_core.transpose(1, 0, 2).reshape(C, BLT).astype(BF16))


def get_nc(repeat=1):
    key = f"nc{repeat}"
    if key not in _cache:
        _cache[key] = _build(repeat)
    return _cache[key]


def make_in_maps(x, layout, heads):
    tab = _tab_const()
    ht = _stage_heads(heads.astype(np.float32))
    lay = np.ascontiguousarray(layout.astype(np.float32))
    return [
        {
            "xc": _stage_x(x[m * B_LOC:(m + 1) * B_LOC]),
            "layout": lay,
            "headsT": ht,
            "tab": tab,
        }
        for m in range(N_CORES)
    ]


def assemble_from_global(g):
    # shard_map-concatenated output [N_CORES*O, BLT] bf16 -> [B, O, T] f32
    g = np.asarray(g).reshape(N_CORES, O, B_LOC, T)
    return np.ascontiguousarray(
        g.transpose(0, 2, 1, 3).reshape(B, O, T)).astype(np.float32)


def assemble_out(res_list):
    # per-core out [O, BLT] bf16 -> full [B, O, T] f32
    outs = []
    for m in range(N_CORES):
        o = np.asarray(res_list[m]["out"])            # [O, BLT] bf16
        o = o.reshape(O, B_LOC, T).transpose(1, 0, 2)  # [B_LOC, O, T]
        outs.append(o.astype(np.float32))
    return np.concatenate(outs, axis=0)


def kernel(x, layout, heads):
    from concourse.bass_utils import run_bass_kernel_spmd

    assert x.shape == (B, C, T) and layout.shape == (C, 2)
    assert heads.shape == (O, D)
    nc = get_nc()
    in_maps = make_in_maps(x, layout, heads)
    res = run_bass_kernel_spmd(nc, in_maps, list(range(N_CORES)))
    return assemble_out(res.results)


# revision 3
# speedup vs baseline: 48.8347x; 2.5644x over previous
"""ChannelMerger v3: v2 + cross-rep software pipelining.

All tile pools are hoisted outside the repeat loop so consecutive kernel
executions pipeline: rep k+1's phase-1 (DVE/ACT/gpsimd embedding + softmax
chain, buffered by rotating tags) runs under rep k's phase-2 (PE matmuls +
DMA), and rep k+1's x loads prefetch during rep k's compute.  PSUM is one
rotating pool of two [128, 2048] tiles shared by scores, sums and phase-2
blocks.  DMA queues: x loads on sync HWDGE, out stores on gpsimd SWDGE,
small phase-1 loads on scalar HWDGE.

Host staging (untimed): x -> bf16 channel-major [C, B_LOC*T] per core;
heads -> bf16 packed headsT [128, 16, O]; layout -> [2, C]; out bf16
[O, B_LOC*T] -> host transpose/cast back to [B, O, T] f32.
"""

import sys

for _p in ("/opt/trn_rl_repo", "/root/.axon_site/_ro/trn_rl_repo"):
    if _p not in sys.path:
        sys.path.append(_p)

import numpy as np
import ml_dtypes

BF16 = ml_dtypes.bfloat16

B, C, T = 64, 270, 2000
O, D = 270, 2048
N_CORES = 8
B_LOC = B // N_CORES
NF = 32
MARGIN = 0.2
WIDTH = 1.4
KC = 8                           # ij chunks per half (8 * 128 = 1024 = D/2)
C_CHUNKS = [(0, 128), (128, 128), (256, 14)]
HALF = 8000                      # x/out processed in two 8000-column halves
PBLK = 2048                      # psum block width (4 banks); matmuls <=512
BLT = B_LOC * T

_cache = {}


def _build(repeat=1):
    import concourse.tile as tile
    from concourse import bacc, mybir

    F32 = mybir.dt.float32
    BF = mybir.dt.bfloat16
    I32 = mybir.dt.int32
    ACT = mybir.ActivationFunctionType
    ALU = mybir.AluOpType
    TWO_PI = float(2.0 * np.pi)

    nc = bacc.Bacc("TRN2", target_bir_lowering=False, debug=False,
                   num_devices=N_CORES)

    xc_ap = nc.dram_tensor("xc", [C, BLT], BF, kind="ExternalInput").ap()
    lay_ap = nc.dram_tensor("lay2", [1, 2 * C], F32, kind="ExternalInput").ap()
    ht_ap = nc.dram_tensor("headsT", [128, 2 * KC, O], BF,
                           kind="ExternalInput").ap()
    tab_ap = nc.dram_tensor("tab", [128, 2 + KC], F32, kind="ExternalInput").ap()
    out_ap = nc.dram_tensor("out", [O, BLT], BF, kind="ExternalOutput").ap()

    with tile.TileContext(nc) as tc:
      with tc.tile_pool(name="const", bufs=1) as cpool, \
           tc.tile_pool(name="expTp", bufs=2) as epool, \
           tc.tile_pool(name="ph1", bufs=1) as wpool, \
           tc.tile_pool(name="xin", bufs=2) as xpool, \
           tc.tile_pool(name="oout", bufs=1) as opool, \
           tc.tile_pool(name="psum", bufs=2, space="PSUM") as mmp:
        for _rep in range(repeat):
            tab = cpool.tile([128, 2 + KC], F32, tag="tab")
            nc.scalar.dma_start(tab[:], tab_ap[:])
            headsT = cpool.tile([128, 2 * KC, O], BF, tag="headsT")
            nc.scalar.dma_start(headsT[:], ht_ap[:])
            lay = cpool.tile([1, 2 * C], F32, tag="lay")
            nc.scalar.dma_start(lay[:], lay_ap[:])

            u_row = cpool.tile([1, C], F32, tag="u_row")
            nc.vector.tensor_scalar(u_row[:], lay[:, 0:C], MARGIN, 1.0 / WIDTH,
                                    ALU.add, ALU.mult)
            v_row = cpool.tile([1, C], F32, tag="v_row")
            nc.vector.tensor_scalar(v_row[:], lay[:, C:2 * C], MARGIN,
                                    1.0 / WIDTH, ALU.add, ALU.mult)
            u_bc = cpool.tile([128, C], F32, tag="u_bc")
            nc.gpsimd.partition_broadcast(u_bc[:], u_row[:])
            v_bc = cpool.tile([128, C], F32, tag="v_bc")
            nc.gpsimd.partition_broadcast(v_bc[:], v_row[:])

            expT = [epool.tile([128, O], BF, tag=f"expT{i}", name=f"expT{i}")
                    for i in range(3)]

            # t2[p, c] = j(p) * v[c]
            t2 = wpool.tile([128, C], F32, tag="t2")
            nc.gpsimd.tensor_scalar(t2[:], v_bc[:], tab[:, 0:1], None, ALU.mult)
            # f_all[p, k, c] = i(k, p) * u[c] + t2[p, c]
            f_all = wpool.tile([128, KC, C], F32, tag="f_all")
            nc.vector.tensor_tensor(
                f_all[:],
                tab[:, 2:2 + KC].unsqueeze(2).broadcast_to([128, KC, C]),
                u_bc[:].unsqueeze(1).broadcast_to([128, KC, C]), ALU.mult)
            nc.vector.tensor_tensor(
                f_all[:], f_all[:],
                t2[:].unsqueeze(1).broadcast_to([128, KC, C]), ALU.add)
            # sin half: f - round(f) in [-.5,.5] on hw (f32->i32 rounds RNE)
            fi = wpool.tile([128, KC, C], I32, tag="fi")
            nc.vector.tensor_copy(fi[:], f_all[:])
            ff = wpool.tile([128, KC, C], F32, tag="ff")
            nc.gpsimd.tensor_copy(ff[:], fi[:])
            fs = wpool.tile([128, KC, C], F32, tag="fs")
            nc.vector.tensor_tensor(fs[:], f_all[:], ff[:], ALU.subtract)
            sin_all = wpool.tile([128, KC, C], BF, tag="sin_all")
            nc.scalar.activation(sin_all[:], fs[:], ACT.Sin, scale=TWO_PI)
            # cos half: frac(f + 0.25); reuse f_all/fi/ff/fs buffers
            g = wpool.tile([128, KC, C], F32, tag="f_all")
            nc.vector.tensor_scalar(g[:], fs[:], 0.25, None, ALU.add)
            gi = wpool.tile([128, KC, C], I32, tag="fi")
            nc.vector.tensor_copy(gi[:], g[:])
            gf = wpool.tile([128, KC, C], F32, tag="ff")
            nc.gpsimd.tensor_copy(gf[:], gi[:])
            gs = wpool.tile([128, KC, C], F32, tag="fs")
            nc.vector.tensor_tensor(gs[:], g[:], gf[:], ALU.subtract)
            cos_all = wpool.tile([128, KC, C], BF, tag="cos_all")
            nc.scalar.activation(cos_all[:], gs[:], ACT.Sin, scale=TWO_PI)

            ones = cpool.tile([128, 1], BF, tag="ones")
            nc.vector.memset(ones[:], 1.0)

            # scores / exp; psum slices come from the shared rotating pool
            for cc, (c0, csz) in enumerate(C_CHUNKS):
                ps = mmp.tile([128, PBLK], F32, tag="mm")
                for k in range(KC):
                    nc.tensor.matmul(ps[:csz, :O], cos_all[:, k, c0:c0 + csz],
                                     headsT[:, k, :], start=(k == 0),
                                     stop=False)
                for k in range(KC):
                    nc.tensor.matmul(ps[:csz, :O], sin_all[:, k, c0:c0 + csz],
                                     headsT[:, KC + k, :], start=False,
                                     stop=(k == KC - 1))
                nc.scalar.activation(expT[cc][:csz, :], ps[:csz, :O], ACT.Exp)
            # denominators as a row; fold normalization into the weights
            ps = mmp.tile([128, PBLK], F32, tag="mm")
            for cc, (c0, csz) in enumerate(C_CHUNKS):
                nc.tensor.matmul(ps[:1, :O], ones[:csz, :], expT[cc][:csz, :],
                                 start=(cc == 0), stop=(cc == 2))
            rrow = wpool.tile([1, O], F32, tag="rrow")
            nc.vector.reciprocal(rrow[:], ps[:1, :O])
            rbc = wpool.tile([128, O], F32, tag="rbc")
            nc.gpsimd.partition_broadcast(rbc[:], rrow[:])
            for cc, (c0, csz) in enumerate(C_CHUNKS):
                nc.vector.tensor_tensor(expT[cc][:csz, :], expT[cc][:csz, :],
                                        rbc[:csz, :], ALU.mult)

            # ---- phase 2: two 8000-column halves, big DMAs ----
            for h in range(2):
                base = h * HALF
                xb = []
                for cc, (c0, csz) in enumerate(C_CHUNKS):
                    xt = xpool.tile([128, HALF], BF, tag=f"x{cc}",
                                    name=f"x{cc}")
                    nc.sync.dma_start(xt[:csz, :],
                                      xc_ap[c0:c0 + csz, base:base + HALF])
                    xb.append(xt)
                for oc, (o0, osz) in enumerate(C_CHUNKS):
                    ot = opool.tile([128, HALF], BF, tag=f"o{oc}",
                                    name=f"o{oc}")
                    for p0 in range(0, HALF, PBLK):
                        psz = min(PBLK, HALF - p0)
                        ph = mmp.tile([128, PBLK], F32, tag="mm")
                        for cc, (c0, csz) in enumerate(C_CHUNKS):
                            for s0 in range(0, psz, 512):
                                ssz = min(512, psz - s0)
                                nc.tensor.matmul(
                                    ph[:osz, s0:s0 + ssz],
                                    expT[cc][:csz, o0:o0 + osz],
                                    xb[cc][:csz, p0 + s0:p0 + s0 + ssz],
                                    start=(cc == 0), stop=(cc == 2))
                        if (oc + p0 // PBLK) % 2 == 0:
                            nc.vector.tensor_copy(ot[:osz, p0:p0 + psz],
                                                  ph[:osz, :psz])
                        else:
                            nc.scalar.activation(ot[:osz, p0:p0 + psz],
                                                 ph[:osz, :psz], ACT.Copy)
                    nc.gpsimd.dma_start(out_ap[o0:o0 + osz, base:base + HALF],
                                        ot[:osz, :])

    nc.compile()
    return nc


def _tab_const():
    p = np.arange(128)
    cols = [(p & 31).astype(np.float32), np.ones(128, np.float32)]
    cols += [((k * 128 + p) >> 5).astype(np.float32) for k in range(KC)]
    return np.stack(cols, axis=1)


def _stage_heads(heads):
    hT = heads.T.astype(BF16)                     # [D, O]
    return np.ascontiguousarray(
        hT.reshape(2 * KC, 128, O).transpose(1, 0, 2))


def _stage_x(x_core):
    return np.ascontiguousarray(
        x_core.transpose(1, 0, 2).reshape(C, BLT).astype(BF16))


def get_nc(repeat=1):
    key = f"nc{repeat}"
    if key not in _cache:
        _cache[key] = _build(repeat)
    return _cache[key]


def make_in_maps(x, layout, heads):
    tab = _tab_const()
    ht = _stage_heads(heads.astype(np.float32))
    lay2 = np.ascontiguousarray(layout.astype(np.float32).T.reshape(1, 2 * C))
    return [
        {
            "xc": _stage_x(x[m * B_LOC:(m + 1) * B_LOC]),
            "lay2": lay2,
            "headsT": ht,
            "tab": tab,
        }
        for m in range(N_CORES)
    ]


def assemble_from_global(g):
    g = np.asarray(g).reshape(N_CORES, O, B_LOC, T)
    return np.ascontiguousarray(
        g.transpose(0, 2, 1, 3).reshape(B, O, T)).astype(np.float32)


def assemble_out(res_list):
    outs = []
    for m in range(N_CORES):
        o = np.asarray(res_list[m]["out"])
        o = o.reshape(O, B_LOC, T).transpose(1, 0, 2)
        outs.append(o.astype(np.float32))
    return np.concatenate(outs, axis=0)


def kernel(x, layout, heads):
    from concourse.bass_utils import run_bass_kernel_spmd

    assert x.shape == (B, C, T) and layout.shape == (C, 2)
    assert heads.shape == (O, D)
    nc = get_nc()
    in_maps = make_in_maps(x, layout, heads)
    res = run_bass_kernel_spmd(nc, in_maps, list(range(N_CORES)))
    return assemble_out(res.results)


# revision 4
# speedup vs baseline: 55.4081x; 1.1346x over previous
"""ChannelMerger v4: v3 + normalization folded into psum-drain copies.

All tile pools are hoisted outside the repeat loop so consecutive kernel
executions pipeline: rep k+1's phase-1 (DVE/ACT/gpsimd embedding + softmax
chain, buffered by rotating tags) runs under rep k's phase-2 (PE matmuls +
DMA), and rep k+1's x loads prefetch during rep k's compute.  PSUM is one
rotating pool of two [128, 2048] tiles shared by scores, sums and phase-2
blocks.  DMA queues: x loads on sync HWDGE, out stores on gpsimd SWDGE,
small phase-1 loads on scalar HWDGE.

Host staging (untimed): x -> bf16 channel-major [C, B_LOC*T] per core;
heads -> bf16 packed headsT [128, 16, O]; layout -> [2, C]; out bf16
[O, B_LOC*T] -> host transpose/cast back to [B, O, T] f32.
"""

import sys

for _p in ("/opt/trn_rl_repo", "/root/.axon_site/_ro/trn_rl_repo"):
    if _p not in sys.path:
        sys.path.append(_p)

import numpy as np
import ml_dtypes

BF16 = ml_dtypes.bfloat16

B, C, T = 64, 270, 2000
O, D = 270, 2048
N_CORES = 8
B_LOC = B // N_CORES
NF = 32
MARGIN = 0.2
WIDTH = 1.4
KC = 8                           # ij chunks per half (8 * 128 = 1024 = D/2)
C_CHUNKS = [(0, 128), (128, 128), (256, 14)]
HALF = 8000                      # x/out processed in two 8000-column halves
PBLK = 2048                      # psum block width (4 banks); matmuls <=512
BLT = B_LOC * T

_cache = {}


def _build(repeat=1):
    import concourse.tile as tile
    from concourse import bacc, mybir

    F32 = mybir.dt.float32
    BF = mybir.dt.bfloat16
    I32 = mybir.dt.int32
    ACT = mybir.ActivationFunctionType
    ALU = mybir.AluOpType
    TWO_PI = float(2.0 * np.pi)

    nc = bacc.Bacc("TRN2", target_bir_lowering=False, debug=False,
                   num_devices=N_CORES)

    xc_ap = nc.dram_tensor("xc", [C, BLT], BF, kind="ExternalInput").ap()
    lay_ap = nc.dram_tensor("lay2", [1, 2 * C], F32, kind="ExternalInput").ap()
    ht_ap = nc.dram_tensor("headsT", [128, 2 * KC, O], BF,
                           kind="ExternalInput").ap()
    tab_ap = nc.dram_tensor("tab", [128, 2 + KC], F32, kind="ExternalInput").ap()
    out_ap = nc.dram_tensor("out", [O, BLT], BF, kind="ExternalOutput").ap()

    with tile.TileContext(nc) as tc:
      with tc.tile_pool(name="const", bufs=1) as cpool, \
           tc.tile_pool(name="expTp", bufs=2) as epool, \
           tc.tile_pool(name="ph1", bufs=1) as wpool, \
           tc.tile_pool(name="xin", bufs=2) as xpool, \
           tc.tile_pool(name="oout", bufs=1) as opool, \
           tc.tile_pool(name="psum", bufs=2, space="PSUM") as mmp:
        for _rep in range(repeat):
            tab = cpool.tile([128, 2 + KC], F32, tag="tab")
            nc.scalar.dma_start(tab[:], tab_ap[:])
            headsT = cpool.tile([128, 2 * KC, O], BF, tag="headsT")
            nc.scalar.dma_start(headsT[:], ht_ap[:])
            lay = cpool.tile([1, 2 * C], F32, tag="lay")
            nc.scalar.dma_start(lay[:], lay_ap[:])

            u_row = cpool.tile([1, C], F32, tag="u_row")
            nc.vector.tensor_scalar(u_row[:], lay[:, 0:C], MARGIN, 1.0 / WIDTH,
                                    ALU.add, ALU.mult)
            v_row = cpool.tile([1, C], F32, tag="v_row")
            nc.vector.tensor_scalar(v_row[:], lay[:, C:2 * C], MARGIN,
                                    1.0 / WIDTH, ALU.add, ALU.mult)
            u_bc = cpool.tile([128, C], F32, tag="u_bc")
            nc.gpsimd.partition_broadcast(u_bc[:], u_row[:])
            v_bc = cpool.tile([128, C], F32, tag="v_bc")
            nc.gpsimd.partition_broadcast(v_bc[:], v_row[:])

            expT = [epool.tile([128, O], BF, tag=f"expT{i}", name=f"expT{i}")
                    for i in range(3)]

            # t2[p, c] = j(p) * v[c]
            t2 = wpool.tile([128, C], F32, tag="t2")
            nc.gpsimd.tensor_scalar(t2[:], v_bc[:], tab[:, 0:1], None, ALU.mult)
            # f_all[p, k, c] = i(k, p) * u[c] + t2[p, c]
            f_all = wpool.tile([128, KC, C], F32, tag="f_all")
            nc.vector.tensor_tensor(
                f_all[:],
                tab[:, 2:2 + KC].unsqueeze(2).broadcast_to([128, KC, C]),
                u_bc[:].unsqueeze(1).broadcast_to([128, KC, C]), ALU.mult)
            nc.vector.tensor_tensor(
                f_all[:], f_all[:],
                t2[:].unsqueeze(1).broadcast_to([128, KC, C]), ALU.add)
            # sin half: f - round(f) in [-.5,.5] on hw (f32->i32 rounds RNE)
            fi = wpool.tile([128, KC, C], I32, tag="fi")
            nc.vector.tensor_copy(fi[:], f_all[:])
            ff = wpool.tile([128, KC, C], F32, tag="ff")
            nc.gpsimd.tensor_copy(ff[:], fi[:])
            fs = wpool.tile([128, KC, C], F32, tag="fs")
            nc.vector.tensor_tensor(fs[:], f_all[:], ff[:], ALU.subtract)
            sin_all = wpool.tile([128, KC, C], BF, tag="sin_all")
            nc.scalar.activation(sin_all[:], fs[:], ACT.Sin, scale=TWO_PI)
            # cos half: frac(f + 0.25); reuse f_all/fi/ff/fs buffers
            g = wpool.tile([128, KC, C], F32, tag="f_all")
            nc.vector.tensor_scalar(g[:], fs[:], 0.25, None, ALU.add)
            gi = wpool.tile([128, KC, C], I32, tag="fi")
            nc.vector.tensor_copy(gi[:], g[:])
            gf = wpool.tile([128, KC, C], F32, tag="ff")
            nc.gpsimd.tensor_copy(gf[:], gi[:])
            gs = wpool.tile([128, KC, C], F32, tag="fs")
            nc.vector.tensor_tensor(gs[:], g[:], gf[:], ALU.subtract)
            cos_all = wpool.tile([128, KC, C], BF, tag="cos_all")
            nc.scalar.activation(cos_all[:], gs[:], ACT.Sin, scale=TWO_PI)

            ones = cpool.tile([128, 1], BF, tag="ones")
            nc.vector.memset(ones[:], 1.0)

            # scores / exp; psum slices come from the shared rotating pool
            for cc, (c0, csz) in enumerate(C_CHUNKS):
                ps = mmp.tile([128, PBLK], F32, tag="mm")
                for k in range(KC):
                    nc.tensor.matmul(ps[:csz, :O], cos_all[:, k, c0:c0 + csz],
                                     headsT[:, k, :], start=(k == 0),
                                     stop=False)
                for k in range(KC):
                    nc.tensor.matmul(ps[:csz, :O], sin_all[:, k, c0:c0 + csz],
                                     headsT[:, KC + k, :], start=False,
                                     stop=(k == KC - 1))
                nc.scalar.activation(expT[cc][:csz, :], ps[:csz, :O], ACT.Exp)
            # denominators per o-chunk as psum columns; normalization is
            # applied later inside the psum-drain copies, so phase-2 matmuls
            # only wait on exp (shortens the PE-gating chain per rep)
            recip = epool.tile([128, 4], F32, tag="recip")
            ps = mmp.tile([128, PBLK], F32, tag="mm")
            for oc, (o0, osz) in enumerate(C_CHUNKS):
                for cc, (c0, csz) in enumerate(C_CHUNKS):
                    nc.tensor.matmul(ps[:osz, oc * 512:oc * 512 + 1],
                                     expT[cc][:csz, o0:o0 + osz],
                                     ones[:csz, :],
                                     start=(cc == 0), stop=(cc == 2))
            for oc, (o0, osz) in enumerate(C_CHUNKS):
                nc.vector.reciprocal(recip[:osz, oc:oc + 1],
                                     ps[:osz, oc * 512:oc * 512 + 1])

            # ---- phase 2: two 8000-column halves, big DMAs ----
            for h in range(2):
                base = h * HALF
                xb = []
                for cc, (c0, csz) in enumerate(C_CHUNKS):
                    xt = xpool.tile([128, HALF], BF, tag=f"x{cc}",
                                    name=f"x{cc}")
                    nc.sync.dma_start(xt[:csz, :],
                                      xc_ap[c0:c0 + csz, base:base + HALF])
                    xb.append(xt)
                for oc, (o0, osz) in enumerate(C_CHUNKS):
                    ot = opool.tile([128, HALF], BF, tag=f"o{oc}",
                                    name=f"o{oc}")
                    for p0 in range(0, HALF, PBLK):
                        psz = min(PBLK, HALF - p0)
                        ph = mmp.tile([128, PBLK], F32, tag="mm")
                        for cc, (c0, csz) in enumerate(C_CHUNKS):
                            for s0 in range(0, psz, 512):
                                ssz = min(512, psz - s0)
                                nc.tensor.matmul(
                                    ph[:osz, s0:s0 + ssz],
                                    expT[cc][:csz, o0:o0 + osz],
                                    xb[cc][:csz, p0 + s0:p0 + s0 + ssz],
                                    start=(cc == 0), stop=(cc == 2))
                        if (oc + p0 // PBLK) % 2 == 0:
                            nc.vector.tensor_scalar(ot[:osz, p0:p0 + psz],
                                                    ph[:osz, :psz],
                                                    recip[:osz, oc:oc + 1],
                                                    None, ALU.mult)
                        else:
                            nc.scalar.activation(ot[:osz, p0:p0 + psz],
                                                 ph[:osz, :psz], ACT.Copy,
                                                 scale=recip[:osz, oc:oc + 1])
                    nc.gpsimd.dma_start(out_ap[o0:o0 + osz, base:base + HALF],
                                        ot[:osz, :])

    nc.compile()
    return nc


def _tab_const():
    p = np.arange(128)
    cols = [(p & 31).astype(np.float32), np.ones(128, np.float32)]
    cols += [((k * 128 + p) >> 5).astype(np.float32) for k in range(KC)]
    return np.stack(cols, axis=1)


def _stage_heads(heads):
    hT = heads.T.astype(BF16)                     # [D, O]
    return np.ascontiguousarray(
        hT.reshape(2 * KC, 128, O).transpose(1, 0, 2))


def _stage_x(x_core):
    return np.ascontiguousarray(
        x_core.transpose(1, 0, 2).reshape(C, BLT).astype(BF16))


def get_nc(repeat=1):
    key = f"nc{repeat}"
    if key not in _cache:
        _cache[key] = _build(repeat)
    return _cache[key]


def make_in_maps(x, layout, heads):
    tab = _tab_const()
    ht = _stage_heads(heads.astype(np.float32))
    lay2 = np.ascontiguousarray(layout.astype(np.float32).T.reshape(1, 2 * C))
    return [
        {
            "xc": _stage_x(x[m * B_LOC:(m + 1) * B_LOC]),
            "lay2": lay2,
            "headsT": ht,
            "tab": tab,
        }
        for m in range(N_CORES)
    ]


def assemble_from_global(g):
    g = np.asarray(g).reshape(N_CORES, O, B_LOC, T)
    return np.ascontiguousarray(
        g.transpose(0, 2, 1, 3).reshape(B, O, T)).astype(np.float32)


def assemble_out(res_list):
    outs = []
    for m in range(N_CORES):
        o = np.asarray(res_list[m]["out"])
        o = o.reshape(O, B_LOC, T).transpose(1, 0, 2)
        outs.append(o.astype(np.float32))
    return np.concatenate(outs, axis=0)


def kernel(x, layout, heads):
    from concourse.bass_utils import run_bass_kernel_spmd

    assert x.shape == (B, C, T) and layout.shape == (C, 2)
    assert heads.shape == (O, D)
    nc = get_nc()
    in_maps = make_in_maps(x, layout, heads)
    res = run_bass_kernel_spmd(nc, in_maps, list(range(N_CORES)))
    return assemble_out(res.results)
